# revision 27
# baseline (speedup 1.0000x reference)
"""Causal GQA self-attention (B=4, T=2048, C=2048, 16 heads / 4 kv-heads,
l2-normalized q,k) on 8 Trainium2 NeuronCores.

Sharding: core pair (2b, 2b+1) handles batch b. Within a pair, queries are
split stride-2 by row parity (parity p takes rows p::2); odd-parity cores
receive x with adjacent rows pair-swapped so one compiled program serves all
8 cores, with the swapped in-chunk key order absorbed into that core's
causal-mask tiles.

Host supplies x already transposed (feature-major) in both fp8 (q/k
projections, DoubleRow) and bf16 (v projection) so the kernel does no PE
transposes at all. Per core:
  k,q   = fp8 DoubleRow projections off the fp8 xT; weights pre-scaled by 64
          (l2norm makes q,k scale-invariant). Query columns are read straight
          from the fp8 xT with a stride-2 access pattern.
  q8,k8 = l2-normalized q,k scaled by 32 and written as fp8 for the scores
          matmuls; k8 carries a zeros slice so scores can run as DoubleRow
          with a zero-padded second contraction slice (contraction is only
          the 128-dim head axis).
  v     = bf16 projection emitted directly in [keys, feat] layout
          (lhsT = xT block), no re-transpose.
  attn  = scores^T fp8-DoubleRow into PSUM, exp on ACT with the 1/(32*32)
          fp8 scaling folded into the 1/sqrt(hd) exp scale, causal masking =
          fp16 0/1 multiply, denominator = single fp16 running-sum tile +
          all-ones matmul (deferred one head so the PE never waits on the
          DVE chain), AV with v as lhsT (fp16), yT normalized by reciprocal.
  out   = Wproj^T consumes yT (bf16), written feature-major; out-projection
          of strip 0 is interleaved into the attention of strip 1 so the PE
          stays busy while ACT works through the exps.
Input DMAs are split across the SP and Pool queues in consumption order so
no projection phase waits on a cold transfer.
Host transposes the [C, 1024] result and scatters rows p::2 of batch b.
"""

import numpy as np
import ml_dtypes

import concourse.bacc as bacc
import concourse.mybir as mybir
import concourse.tile as tile
from concourse.bass_utils import run_bass_kernel_spmd

B, T, C = 4, 2048, 2048
NH, NKV, HD = 16, 4, 128
KV = 512            # k (and v) projection width
P = 128
SCALE = 1.0 / float(np.sqrt(HD))
N_CORES = 8
W_SCALE = 64.0      # host pre-scale on Wq/Wk so fp8 values are normal-range
QK_SCALE = 32.0     # on-device scale on normalized q,k for fp8 scores

F32 = mybir.dt.float32
BF16 = mybir.dt.bfloat16
FP16 = mybir.dt.float16
F8 = mybir.dt.float8e4
Exp = mybir.ActivationFunctionType.Exp
Sqrt = mybir.ActivationFunctionType.Sqrt
Square = mybir.ActivationFunctionType.Square
MUL = mybir.AluOpType.mult
ADD = mybir.AluOpType.add
DR = mybir.MatmulPerfMode.DoubleRow

NST = 4             # four 512-row T strips
NCC = 16            # C contraction chunks of 128
NCP = NCC // 2      # 8 DoubleRow contraction pairs
NM_K = KV // P      # 4  (kdim / vdim output chunks)
NM_Q = C // P       # 16 (qdim output chunks)
LQ = T // 2         # 1024 local query rows per core
SS_W = 512          # local queries per out-proj strip
EXP_SCALE = SCALE / (QK_SCALE * QK_SCALE)


def build():
    nc = bacc.Bacc("TRN2", target_bir_lowering=False, debug=False,
                   num_devices=N_CORES)
    # x transposed on host: fp8 strips (t-major) and bf16 t-blocks
    x8d = nc.declare_dram_parameter("x8d", [NST, P, NCC, 512], F8,
                                    isOutput=False)
    xtd = nc.declare_dram_parameter("xtd", [NCC, P, NCC, P], BF16,
                                    isOutput=False)
    # DoubleRow-interleaved fp8 weights: [p, m, j, i, f] =
    # W[(2j+i)*128+p, m*128+f] * W_SCALE
    wq8 = nc.declare_dram_parameter("wq8", [P, NM_Q, NCP, 2, P], F8,
                                    isOutput=False)
    wk8 = nc.declare_dram_parameter("wk8", [P, NM_K, NCP, 2, P], F8,
                                    isOutput=False)
    wvd = nc.declare_dram_parameter("wvd", [P, NCC, KV], BF16, isOutput=False)
    wpd = nc.declare_dram_parameter("wpd", [4, P, NH, 512], BF16,
                                    isOutput=False)
    masks = nc.declare_dram_parameter("masks", [P, 2, P], FP16,
                                      isOutput=False)
    ones_in = nc.declare_dram_parameter("onesb", [P, P], BF16, isOutput=False)
    onesh_in = nc.declare_dram_parameter("onesh", [P, P], FP16, isOutput=False)
    out = nc.declare_dram_parameter("out", [C, LQ], F32, isOutput=True)

    with tile.TileContext(nc) as tc:
        with (
            tc.tile_pool(name="cst", bufs=1) as cst,
            tc.tile_pool(name="q8", bufs=1) as p_q8,
            tc.tile_pool(name="k8", bufs=1) as p_k8,
            tc.tile_pool(name="vsb", bufs=1) as p_v,
            tc.tile_pool(name="wv", bufs=1) as p_wv,
            tc.tile_pool(name="xt", bufs=4) as p_xt,
        ):
            ones_bf = cst.tile([P, P], BF16)
            ones_h = cst.tile([P, P], FP16)
            dmask = cst.tile([P, 2, P], FP16)

            q8_sp = [p_q8.tile([P, NH + 1, 512], F8, name=f"q8_{i}")
                     for i in range(2)]                  # 8.5 KB/part each
            k8_sb = p_k8.tile([P, NKV + 1, T], F8)       # 10 KB/part
            v_sb = p_v.tile([P, NCC, KV], FP16)          # 16 KB/part

            # zero pads: k8 slice NKV is the DoubleRow zero slice; q8 slice NH
            # is the don't-care pair slice for head 15 (must be finite)
            nc.gpsimd.memset(k8_sb[:, NKV, :], 0.0)
            nc.gpsimd.memset(q8_sp[0][:, NH, :], 0.0)
            nc.gpsimd.memset(q8_sp[1][:, NH, :], 0.0)
            warm = cst.tile([P, 1], F32)

            # --------- Phases K (k proj), V (v proj), Q (q proj) ---------
            with (
                tc.tile_pool(name="x8", bufs=1) as p_x8,
                tc.tile_pool(name="wk", bufs=1) as p_wk,
                tc.tile_pool(name="wq", bufs=1) as p_wq,
                tc.tile_pool(name="kT", bufs=2) as p_kT,
                tc.tile_pool(name="qT", bufs=1) as p_qT,
                tc.tile_pool(name="sq", bufs=2) as p_sq,
                tc.tile_pool(name="nrm", bufs=1) as p_nrm,
                tc.tile_pool(name="pa", bufs=3, space="PSUM") as ps_a,
                tc.tile_pool(name="pss", bufs=2, space="PSUM") as ps_s,
            ):
                # SP queue, in consumption order: x8 strip 0, first k
                # weight chunk, consts, remaining strips, v weights
                x8_sb = p_x8.tile([P, NCC, T], F8)       # 32 KB/part
                wk_t = p_wk.tile([P, NM_K, NCP, 2, P], F8)
                nc.sync.dma_start(x8_sb[:, 0:8, 0:512], x8d[0, :, 0:8, :])
                nc.sync.dma_start(wk_t[:, 0], wk8[:, 0])
                nc.sync.dma_start(ones_bf[:], ones_in[:])
                nc.sync.dma_start(x8_sb[:, 8:NCC, 0:512],
                                  x8d[0, :, 8:NCC, :])
                nc.sync.dma_start(x8_sb[:, :, 512:1024], x8d[1, :, :, :])
                for m in range(1, NM_K):
                    nc.sync.dma_start(wk_t[:, m], wk8[:, m])
                for st in range(2, NST):
                    nc.sync.dma_start(
                        x8_sb[:, :, st * 512:(st + 1) * 512],
                        x8d[st, :, :, :])
                wq_t = p_wq.tile([P, NM_Q, NCP, 2, P], F8)   # 32 KB/part
                for mg in range(4):
                    nc.sync.dma_start(wq_t[:, 4 * mg:4 * mg + 4],
                                      wq8[:, 4 * mg:4 * mg + 4])
                wv_t = p_wv.tile([P, NCC, KV], BF16)
                nc.sync.dma_start(wv_t[:], wvd[:])
                xts = []
                for tb in range(4):
                    xt_t = p_xt.tile([P, NCC, P], BF16, tag="xt")
                    nc.sync.dma_start(xt_t[:], xtd[tb, :, :, :])
                    xts.append(xt_t)
                # Pool queue: the fp16 ones and the attn masks
                nc.gpsimd.dma_start(ones_h[:], onesh_in[:])
                nc.gpsimd.dma_start(dmask[:], masks[:])

                # ---- Phase K ----
                for st in range(NST):
                    tsl = slice(st * 512, (st + 1) * 512)
                    kT_t = p_kT.tile([P, NM_K, 512], BF16, tag="kT")
                    ssqk = ps_s.tile([P, 512], F32, tag="ssq")
                    for m in range(NM_K):
                        pk = ps_a.tile([P, 512], F32, tag="acc")
                        for j in range(NCP):
                            nc.tensor.matmul(
                                pk[:], wk_t[:, m, j, :, :],
                                x8_sb[:, 2 * j:2 * j + 2, tsl],
                                start=(j == 0), stop=(j == NCP - 1),
                                perf_mode=DR)
                        nc.scalar.copy(out=kT_t[:, m, :], in_=pk[:])
                        sq = p_sq.tile([P, 512], BF16, tag="sq")
                        nc.vector.tensor_tensor(
                            sq[:], kT_t[:, m, :], kT_t[:, m, :], MUL)
                        nc.tensor.matmul(
                            ssqk[:], ones_bf[:], sq[:],
                            start=(m == 0), stop=(m == NM_K - 1))
                    # nrm = |k| / QK_SCALE, so rk = QK_SCALE / |k|
                    nrm = p_nrm.tile([P, 512], F32, tag="nrm")
                    nc.scalar.activation(nrm[:], ssqk[:], Sqrt,
                                         scale=1.0 / (QK_SCALE * QK_SCALE))
                    rk = p_nrm.tile([P, 512], F32, tag="rk")
                    nc.vector.reciprocal_approx_fast(rk[:], nrm[:])
                    rk16 = p_sq.tile([P, 512], BF16, tag="rk16")
                    nc.vector.tensor_copy(rk16[:], rk[:])
                    nc.vector.tensor_tensor(
                        k8_sb[:, 0:NKV, tsl], kT_t[:, :, :],
                        rk16[:, None, :].to_broadcast([P, NM_K, 512]), MUL)

                def v_block(tb, pool, tag="acc"):
                    xt_t = xts[tb]
                    if tb + 4 < NCC:
                        nxt = p_xt.tile([P, NCC, P], BF16, tag="xt")
                        nc.sync.dma_start(nxt[:], xtd[tb + 4, :, :, :])
                        xts.append(nxt)
                    pv = pool.tile([P, 512], F32, tag=tag)
                    for cc in range(NCC):
                        nc.tensor.matmul(
                            pv[:], xt_t[:, cc, :], wv_t[:, cc, :],
                            start=(cc == 0), stop=(cc == NCC - 1))
                    nc.scalar.copy(out=v_sb[:, tb, :], in_=pv[:])

                # ---- Phase Q (before V: the V matmuls then hide the whole
                # q-norm / fp8-cast / exp-table-load tail) ----
                qT_sb = p_qT.tile([P, NM_Q, LQ], BF16)
                for sp in range(2):
                    qsl = slice(sp * 512, (sp + 1) * 512)
                    # local queries sp*512.. are x8 columns 2*l (host
                    # pair-swap puts this core's parity at even columns)
                    xq = slice(sp * 1024, (sp + 1) * 1024, 2)
                    ssq = ps_s.tile([P, 512], F32, tag="ssq")
                    for m in range(NM_Q):
                        pq = ps_a.tile([P, 512], F32, tag="acc")
                        for j in range(NCP):
                            nc.tensor.matmul(
                                pq[:], wq_t[:, m, j, :, :],
                                x8_sb[:, 2 * j:2 * j + 2, xq],
                                start=(j == 0), stop=(j == NCP - 1),
                                perf_mode=DR)
                        nc.scalar.copy(out=qT_sb[:, m, qsl], in_=pq[:])
                        sq = p_sq.tile([P, 512], BF16, tag="sq")
                        if sp == 0:
                            nc.vector.tensor_tensor(
                                sq[:], qT_sb[:, m, qsl], qT_sb[:, m, qsl],
                                MUL)
                        else:
                            # DVE is draining the sp0 q8 casts; square on
                            # ACT so the ssq matmul is never queued behind
                            nc.scalar.activation(sq[:], pq[:], Square)
                        nc.tensor.matmul(
                            ssq[:], ones_bf[:], sq[:],
                            start=(m == 0), stop=(m == NM_Q - 1))
                    nrm = p_nrm.tile([P, 512], F32, tag="nrm")
                    nc.scalar.activation(nrm[:], ssq[:], Sqrt,
                                         scale=1.0 / (QK_SCALE * QK_SCALE))
                    rq = p_nrm.tile([P, 512], F32, tag="rq")
                    nc.vector.reciprocal_approx_fast(rq[:], nrm[:])
                    rq16 = p_sq.tile([P, 512], BF16, tag="rq16")
                    nc.vector.tensor_copy(rq16[:], rq[:])
                    for mg in range(4):
                        nc.vector.tensor_tensor(
                            q8_sp[sp][:, 4 * mg:4 * mg + 4, :],
                            qT_sb[:, 4 * mg:4 * mg + 4, qsl],
                            rq16[:, None, :].to_broadcast([P, 4, 512]), MUL)
                    if sp == 1:
                        # pull the exp act-table load under the q-proj tail
                        nc.scalar.activation(warm[:], ones_bf[:, 0:1], Exp)

                # ---- Phase V (t-blocks 12..15 feed only sub-strip 3 and
                # run as PE filler inside the first attention phase) ----
                for tb in range(12):
                    v_block(tb, ps_a)

            # -------- Phase A: attention with interleaved out-proj --------
            with (
                tc.tile_pool(name="e", bufs=8) as p_e,
                tc.tile_pool(name="acc", bufs=3) as p_acc,
                tc.tile_pool(name="rd", bufs=3) as p_rd,
                tc.tile_pool(name="yT", bufs=1) as p_yT,
                tc.tile_pool(name="wpt", bufs=1) as p_wp,
                tc.tile_pool(name="osb", bufs=2) as p_o,
                tc.tile_pool(name="ps_s", bufs=2, space="PSUM") as ps_s,
                tc.tile_pool(name="ps_y", bufs=3, space="PSUM") as ps_y,
                tc.tile_pool(name="ps_d", bufs=1, space="PSUM") as ps_d,
            ):
                def flush_carry(carry):
                    # the deferred diagonal AV group of the previous head:
                    # emitting it under the next head's first scores hides
                    # the exp->mask latency behind real PE work. The band is
                    # trimmed: chunks d2,d3 only touch the upper 128 queries.
                    e_diag, py_c, nk_c, g_c = carry
                    d = nk_c - 4
                    gsl = slice(g_c * P, (g_c + 1) * P)
                    nc.tensor.matmul(
                        py_c[:], v_sb[:, d, gsl], e_diag[:, 0, :],
                        start=(nk_c == 4), stop=False)
                    for i in (2, 3):
                        nc.tensor.matmul(
                            py_c[:, 128:256], v_sb[:, d + i, gsl],
                            e_diag[:, i, 128:256],
                            start=False, stop=False, skip_group_check=True)
                    nc.tensor.matmul(
                        py_c[:], v_sb[:, d + 1, gsl], e_diag[:, 1, :],
                        start=False, stop=True)

                def att_head(ssf, h, carry):
                    """Scores/exp/AV for one head. The diagonal AV group and
                    the denominator tail are both deferred one head (two-stage
                    software pipeline) so the PE never waits on ACT/DVE."""
                    g = h // 4
                    nk = 4 * (ssf + 1)
                    q8_t = q8_sp[ssf // 2]
                    q0 = (ssf % 2) * 256
                    qsl = slice(q0, q0 + 256)
                    qsl_up = slice(q0 + 128, q0 + 256)
                    py = ps_y.tile([P, 256], F32, tag="y")
                    acc = p_acc.tile([P, 256], FP16, tag="acc")
                    e_prev = None
                    for kg in range(nk // 4):
                        diag = kg == nk // 4 - 1
                        psc = ps_s.tile([P, 4, 256], F32, tag="s")
                        for i in range(4):
                            kc = 4 * kg + i
                            # diagonal chunks d2,d3 only reach the upper 128
                            # queries of the 256-query sub-strip
                            up = diag and i >= 2
                            nc.tensor.matmul(
                                psc[:, i, 128:256] if up else psc[:, i, :],
                                k8_sb[:, g:NKV + 1:NKV - g,
                                      kc * P:(kc + 1) * P],
                                q8_t[:, h:h + 2, qsl_up if up else qsl],
                                start=True, stop=True, perf_mode=DR)
                        if kg == 0 and carry is not None:
                            flush_carry(carry)
                        if e_prev is not None:
                            for i in range(4):
                                kc = 4 * (kg - 1) + i
                                nc.tensor.matmul(
                                    py[:],
                                    v_sb[:, kc, g * P:(g + 1) * P],
                                    e_prev[:, i, :],
                                    start=(kc == 0), stop=False)
                        e = p_e.tile([P, 4, 256], FP16, tag="e")
                        if diag:
                            nc.scalar.activation(e[:, 0:2, :],
                                                 psc[:, 0:2, :], Exp,
                                                 scale=EXP_SCALE)
                            nc.scalar.activation(e[:, 2:4, 128:256],
                                                 psc[:, 2:4, 128:256], Exp,
                                                 scale=EXP_SCALE)
                            nc.gpsimd.tensor_tensor(
                                e[:, 0:2, 0:128], e[:, 0:2, 0:128],
                                dmask[:], MUL)
                            nc.gpsimd.tensor_tensor(
                                e[:, 2:4, 128:256], e[:, 2:4, 128:256],
                                dmask[:], MUL)
                        else:
                            nc.scalar.activation(e[:], psc[:], Exp,
                                                 scale=EXP_SCALE)
                        # denominator: single [P, 256] fp16 running sum
                        if kg == 0:
                            nc.vector.tensor_tensor(
                                acc[:], e[:, 0, :], e[:, 1, :], ADD)
                        else:
                            for i in (0, 1):
                                nc.vector.tensor_tensor(
                                    acc[:], acc[:], e[:, i, :], ADD)
                        for i in (2, 3):
                            sl = slice(128, 256) if diag else slice(0, 256)
                            nc.vector.tensor_tensor(
                                acc[:, sl], acc[:, sl], e[:, i, sl], ADD)
                        e_prev = e
                    return acc, py, (e_prev, py, nk, g)

                def att_tail(state):
                    acc, py, h, ssf = state
                    pden = ps_d.tile([P, 256], F32, tag="d")
                    nc.tensor.matmul(pden[:], ones_h[:], acc[:],
                                     start=True, stop=True)
                    rden = p_rd.tile([P, 256], F32, tag="rd")
                    nc.vector.reciprocal_approx_fast(rden[:], pden[:])
                    nc.vector.tensor_tensor(yT_t[ssf][:, h, :], py[:],
                                            rden[:], MUL)

                # per-sub-strip yT tiles (256 queries each) so out-proj
                # units never tile-alias with later ymult writes; resident
                # out-proj weights, loaded while the light passes run
                yT_t = [p_yT.tile([P, NH, 256], BF16, name=f"yTt{i}")
                        for i in range(4)]
                wp_ts = [p_wp.tile([P, NH, 512], BF16, name=f"wpt{i}")
                         for i in range(4)]
                for og in range(4):
                    nc.sync.dma_start(wp_ts[og][:], wpd[og, :, :, :])

                def og_unit(ssf, og, j):
                    # eighth of an out-proj og block: 1 j-chunk x 256
                    # queries of one sub-strip. Fine granularity keeps the
                    # interleave with attention smooth.
                    o_sb = p_o.tile([P, 256], F32, tag="o")
                    po = ps_s.tile([P, 256], F32, tag="s")
                    for hh in range(NH):
                        nc.tensor.matmul(
                            po[:], wp_ts[og][:, hh, j * P:(j + 1) * P],
                            yT_t[ssf][:, hh, :],
                            start=(hh == 0), stop=(hh == NH - 1))
                    nc.scalar.copy(out=o_sb[:], in_=po[:])
                    nc.sync.dma_start(
                        out.rearrange("(og j p) q -> p og j q", p=P,
                                      j=4)[:, og, j,
                                           ssf * 256:(ssf + 1) * 256],
                        o_sb[:])

                # mixed-sub-strip pass order: the exp load per pass scales
                # with the sub-strip index (1x..4x), so pairing light ssf0/1
                # with heavy ssf2/3 passes keeps ACT, DVE and PE all busy;
                # out-proj units of finished sub-strips fill the rest
                st = {"pending": None, "carry": None}

                def run_pass(ssf, h):
                    acc, py, st["carry"] = att_head(ssf, h, st["carry"])
                    if st["pending"] is not None:
                        att_tail(st["pending"])
                    st["pending"] = (acc, py, h, ssf)

                ogq = []
                st["ui"] = 0

                def og_push(ssf):
                    for og in range(4):
                        for j in range(4):
                            ogq.append((ssf, og, j))

                def og_pop(n):
                    while n > 0 and st["ui"] < len(ogq):
                        og_unit(*ogq[st["ui"]])
                        st["ui"] += 1
                        n -= 1

                # ssf0 passes lead (they only need the earliest data) with
                # ssf2 staggered 4 heads behind; v t-blocks 12..15 and then
                # ready out-proj units keep the PE fed through the tail
                for h in range(4):
                    run_pass(0, h)
                for h in range(4, NH):
                    run_pass(0, h)
                    run_pass(2, h - 4)
                    if (h - 4) % 3 == 1:
                        v_block(12 + (h - 4) // 3, ps_s, tag="s")
                og_push(0)
                for h in range(NH - 4, NH):
                    run_pass(2, h)
                    og_pop(2)
                og_push(2)
                for h in range(NH):
                    run_pass(1, h)
                    og_pop(1)
                    run_pass(3, h)
                    og_pop(1)
                og_push(1)
                flush_carry(st["carry"])
                att_tail(st["pending"])
                og_push(3)
                og_pop(len(ogq))

    nc.compile()
    return nc


_NC = None


def _get_nc():
    global _NC
    if _NC is None:
        _NC = build()
    return _NC


def _make_masks(p: int) -> np.ndarray:
    # diagonal-band mask for a 128-query half sub-strip against its two
    # partially-valid key chunks; the band is self-similar, so the same
    # [P, 2, 128] pattern serves the lower half (chunks d0,d1) and the
    # upper half (chunks d2,d3) of every sub-strip
    k = np.arange(P)[:, None, None]
    j = np.arange(2)[None, :, None]
    qq = np.arange(P)[None, None, :]
    kp = k if p == 0 else (k ^ 1)
    valid = (2 * qq + p) >= (128 * j + kp)
    return valid.astype(np.float16)


def _dr_weights(w: np.ndarray, nm: int) -> np.ndarray:
    # [p, m, j, i, f] = w[(2j+i)*128+p, m*128+f] * W_SCALE, cast fp8e4m3
    w5 = (w * W_SCALE).reshape(NCP, 2, P, nm, P).transpose(2, 3, 0, 1, 4)
    return np.ascontiguousarray(w5.astype(ml_dtypes.float8_e4m3fn))


def kernel(x, Wq, Wkv, Wproj):
    x = np.asarray(x, dtype=np.float32)
    Wq = np.asarray(Wq, dtype=np.float32)
    Wkv = np.asarray(Wkv, dtype=np.float32)
    Wproj = np.asarray(Wproj, dtype=np.float32)

    wq8 = _dr_weights(Wq, NM_Q)
    wk8 = _dr_weights(Wkv[:, :KV], NM_K)
    # wvd[p, cc, f] = Wv[cc*128+p, f]
    wvd = np.ascontiguousarray(
        Wkv[:, KV:].reshape(NCC, P, KV).transpose(1, 0, 2)
    ).astype(ml_dtypes.bfloat16)
    # wpd[og, p, hh, f] = Wproj[hh*128+p, og*512+f]
    wpd = np.ascontiguousarray(
        Wproj.reshape(NH, P, 4, 512).transpose(2, 1, 0, 3)
    ).astype(ml_dtypes.bfloat16)
    onesb = np.ones((P, P), dtype=ml_dtypes.bfloat16)
    onesh = np.ones((P, P), dtype=np.float16)
    masks_by_p = [_make_masks(0), _make_masks(1)]

    in_maps = []
    for c in range(N_CORES):
        b, p = c // 2, c % 2
        if p == 0:
            xb_c = x[b]
        else:
            # pair-swap rows (2i <-> 2i+1): the program's fixed even-column
            # query gather then selects the odd rows, every key row is still
            # present, and the swapped in-chunk key order is absorbed into
            # this core's mask data.
            xb_c = x[b].reshape(T // 2, 2, C)[:, ::-1, :].reshape(T, C)
        xT_c = np.ascontiguousarray(xb_c.T)              # [C, T] f32
        # x8d[st, p, cc, t'] = xT[cc*128+p, st*512+t']
        x8_c = np.ascontiguousarray(
            xT_c.reshape(NCC, P, NST, 512).transpose(2, 1, 0, 3)
        ).astype(ml_dtypes.float8_e4m3fn)
        # xtd[tb, p, cc, tl] = xT[cc*128+p, tb*128+tl]
        xt_c = np.ascontiguousarray(
            xT_c.reshape(NCC, P, NCC, P).transpose(2, 1, 0, 3)
        ).astype(ml_dtypes.bfloat16)
        in_maps.append({
            "x8d": x8_c, "xtd": xt_c,
            "wq8": wq8, "wk8": wk8, "wvd": wvd, "wpd": wpd,
            "masks": masks_by_p[p],
            "onesb": onesb, "onesh": onesh,
        })

    nc = _get_nc()
    res = run_bass_kernel_spmd(nc, in_maps, list(range(N_CORES)),
                               trace=False)

    result = np.empty((B, T, C), dtype=np.float32)
    for c in range(N_CORES):
        b, p = c // 2, c % 2
        result[b, p::2, :] = res.results[c]["out"].T
    return result
